# revision 39
# baseline (speedup 1.0000x reference)
"""Causal self-attention on 8 trn2 NeuronCores.

Sharding: core c -> (batch b = c // 4, head-group g = c % 4). Each core
computes 4 of the 16 heads for one batch element plus its slice of the
output projection; the host sums the 4 partial projections per batch and
adds the constant (bv @ Wp.T + bp) term exactly.

Kernel structure (per core), streamed over 4 query chunks of 512:
  - Q/K/V projections as fp8e4 DoubleRow matmuls with hi+lo residual
    splits of both x and W (3 accumulation terms; quantization error
    ~0.05%), contraction 256/step.
  - Scores s = k.T q in fp8e4 DoubleRow ([keys, queries] orientation,
    dh packed 32x2), causal mask added in PSUM via an identity matmul,
    exp on the Activation engine straight out of PSUM into bf16 SBUF.
  - attn@v flipped: e is the stationary operand, v (with a trailing
    ones column that accumulates the softmax denominator l) is moving;
    PSUM rows are queries so 1/l is a per-partition scalar folded into
    the eviction tensor_scalar op.
  - y transposed via the DMA xbar (16x128 tiles) into [ydim, t] layout,
    then the output projection in bf16; out partials stored bf16.
"""

import numpy as np
import ml_dtypes

import concourse.bass as bass
import concourse.mybir as mybir
import concourse.tile as tile
from concourse.bass_utils import run_bass_kernel_spmd

B = 2
T = 2048
C = 1024
H = 16
DH = 64
NCORES = 8
GROUPS = 4            # head groups (tensor parallel)
HPG = H // GROUPS     # heads per group = 4
DG = HPG * DH         # head-group width = 256
CHUNK = 512           # query-chunk size
NCHUNK = T // CHUNK   # 4
KO2 = C // 256        # DoubleRow contraction steps for the projections
NKT = T // 128        # key tiles
F32 = mybir.dt.float32
F32R = mybir.dt.float32r
BF16 = mybir.dt.bfloat16
F8 = mybir.dt.float8e4
NPF8 = ml_dtypes.float8_e4m3
NPBF16 = ml_dtypes.bfloat16
MASK_NEG = -1e30

SX = 16.0             # x fp8 scale
SW = 256.0            # weight fp8 scale
SQ = 4.0              # q/k fp8 store scale
DR = mybir.MatmulPerfMode.DoubleRow


def _patch_tile_drain():
    """This walrus build lowers Drain/NOP to a CTRL with a single sync-wait
    slot; TileContext's kernel-tail drain accumulates one wait per live
    semaphore and fails codegen. Split the waits across single-wait NOPs."""
    import bass_rust
    from concourse.tile import TileContext

    def _drain_and_barrier_split(self, tick_clock, wait_clock):
        probe = self.nc.sync.nop()
        wait_clock.add_sem_waits(
            probe.ins, tile.ScopedClock({None: tick_clock.global_clock})
        )
        waits = list(probe.ins.sync_info.on_wait or [])
        probe.ins.sync_info.on_wait = []
        engines = [self.nc.sync, self.nc.tensor, self.nc.vector,
                   self.nc.scalar, self.nc.gpsimd]
        for i, w in enumerate(waits):
            n = engines[i % len(engines)].nop()
            if n.ins.sync_info is None:
                n.ins.sync_info = bass_rust.SyncInfo(on_wait=[w], on_update=[])
            else:
                n.ins.sync_info.on_wait = [w]
        self.nc.sync.drain()
        self.nc.all_engine_barrier()
        assert self.sems is not None
        popped = self.nc._tile_sem_poison_stack.pop()
        assert popped is self._sem_poison
        self.nc.clear_and_free_semaphores(list(self.sems.allocated().values()))
        self.nc.all_engine_barrier()

    TileContext._drain_and_barrier = _drain_and_barrier_split

    import json as _json

    import concourse.bass2jax as bass2jax
    import concourse.bass_utils as bass_utils

    if getattr(bass_utils.compile_bir_kernel, "_wait_split", False):
        return

    _orig_compile = bass_utils.compile_bir_kernel

    def _split_multi_waits(bir_json):
        m = _json.loads(bir_json)
        counter = 0
        changed = False
        for fn in m["functions"]:
            for blk in fn["blocks"]:
                new_insts = []
                for inst in blk["instructions"]:
                    si = inst.get("sync_info")
                    waits = (si or {}).get("on_wait") or []
                    sem_waits = [w for w in waits if w.get("sync_type") == "semaphore"]
                    if len(waits) > 1 and len(sem_waits) == len(waits):
                        changed = True
                        for w in waits[:-1]:
                            counter += 1
                            new_insts.append({
                                "name": f"I-wsplit{counter}",
                                "opcode": "NoOp",
                                "engine": inst["engine"],
                                "ins": [],
                                "outs": [],
                                "sync_info": {"on_wait": [w], "on_update": []},
                            })
                        si["on_wait"] = [waits[-1]]
                    new_insts.append(inst)
                blk["instructions"] = new_insts
        if not changed:
            return bir_json
        return _json.dumps(m).encode()

    def _compile_bir_kernel_split(bir_json, tmpdir, neff_name="file.neff"):
        return _orig_compile(_split_multi_waits(bir_json), tmpdir, neff_name=neff_name)

    _compile_bir_kernel_split._wait_split = True
    bass_utils.compile_bir_kernel = _compile_bir_kernel_split
    bass2jax.compile_bir_kernel = _compile_bir_kernel_split


def build_kernel():
    _patch_tile_drain()
    nc = bass.Bass(target_bir_lowering=False, trn_type="TRN2")

    # hi/lo fp8 operand pairs; layouts are DoubleRow-packed on the host:
    # contraction index c = ko*256 + slot*128 + p.
    xh = nc.dram_tensor("xh", [NCHUNK, 128, 2, KO2, CHUNK], F8, kind="ExternalInput")
    xl = nc.dram_tensor("xl", [NCHUNK, 128, 2, KO2, CHUNK], F8, kind="ExternalInput")
    wqh = nc.dram_tensor("wqh", [128, 2, KO2, DG], F8, kind="ExternalInput")
    wql = nc.dram_tensor("wql", [128, 2, KO2, DG], F8, kind="ExternalInput")
    wkh = nc.dram_tensor("wkh", [128, 2, KO2, DG], F8, kind="ExternalInput")
    wkl = nc.dram_tensor("wkl", [128, 2, KO2, DG], F8, kind="ExternalInput")
    wvh = nc.dram_tensor("wvh", [128, 2, KO2, DG], F8, kind="ExternalInput")
    wvl = nc.dram_tensor("wvl", [128, 2, KO2, DG], F8, kind="ExternalInput")
    wpt = nc.dram_tensor("wpt", [128, 2, C], BF16, kind="ExternalInput")
    bq4 = nc.dram_tensor("bq4", [128, 2], F32, kind="ExternalInput")
    bk4 = nc.dram_tensor("bk4", [128, 2], F32, kind="ExternalInput")
    out = nc.dram_tensor("out", [NKT, 128, C], BF16, kind="ExternalOutput")

    from contextlib import ExitStack

    with tile.TileContext(nc) as tc, ExitStack() as ctx:
        from concourse.masks import make_identity

        const = ctx.enter_context(tc.tile_pool(name="const", bufs=1))
        xpool = ctx.enter_context(tc.tile_pool(name="xp", bufs=4))
        persist = ctx.enter_context(tc.tile_pool(name="persist", bufs=1))
        epool = ctx.enter_context(tc.tile_pool(name="ep", bufs=5))
        ypool = ctx.enter_context(tc.tile_pool(name="yp", bufs=2))
        ytpool = ctx.enter_context(tc.tile_pool(name="ytp", bufs=4))
        opool = ctx.enter_context(tc.tile_pool(name="op", bufs=3))
        small = ctx.enter_context(tc.tile_pool(name="sm", bufs=6))
        ps_big = ctx.enter_context(tc.tile_pool(name="psb", bufs=2, space="PSUM"))
        ps_y = ctx.enter_context(tc.tile_pool(name="psy", bufs=2, space="PSUM"))
        ps_o = ctx.enter_context(tc.tile_pool(name="pso", bufs=2, space="PSUM"))

        _x_tiles = {}

        def prefetch_x(n):
            if n not in _x_tiles and n < NCHUNK:
                th = xpool.tile([128, 2, KO2, CHUNK], F8, tag="x", name=f"xh{n}")
                nc.sync.dma_start(th[:], xh[n])
                tl = xpool.tile([128, 2, KO2, CHUNK], F8, tag="x", name=f"xl{n}")
                nc.sync.dma_start(tl[:], xl[n])
                _x_tiles[n] = (th, tl)

        def load_x(n):
            prefetch_x(n)
            return _x_tiles.pop(n)

        # ---- constants ----  (x chunk 0 is prefetched right after the wq
        # pair so the first projection matmul can start ~2.5us in)
        wq_sb, wk_sb, wv_sb = [], [], []
        _w_srcs = ((wq_sb, wqh, wql), (wk_sb, wkh, wkl), (wv_sb, wvh, wvl))
        _w_tiles = []
        for wn, (dst, hi, lo) in enumerate(_w_srcs):
            for hl, w_dram in enumerate((hi, lo)):
                t = const.tile([128, 2, KO2, DG], F8, name=f"w{wn}_{hl}")
                _w_tiles.append((t, w_dram))
                dst.append(t)
        _order = [0, 2, 1, 3, 4, 5]        # wq-hi, wk-hi, wq-lo, wk-lo, wv
        nc.sync.dma_start(_w_tiles[0][0][:], _w_tiles[0][1][:])   # wq hi
        nc.sync.dma_start(_w_tiles[2][0][:], _w_tiles[2][1][:])   # wk hi
        prefetch_x(0)
        for wi in (1, 3, 4, 5):
            t, w_dram = _w_tiles[wi]
            nc.sync.dma_start(t[:], w_dram[:])
        bq_sb = const.tile([128, 2], F32)
        nc.sync.dma_start(bq_sb[:], bq4[:])
        bk_sb = const.tile([128, 2], F32)
        nc.sync.dma_start(bk_sb[:], bk4[:])
        wpt_sb = const.tile([128, 2, C], BF16)
        nc.sync.dma_start(wpt_sb[:], wpt[:])

        ident = const.tile([128, 128], BF16)
        make_identity(nc, ident)
        ones_row = const.tile([1, 128], BF16)
        nc.vector.memset(ones_row[:], 1.0)
        zero_row = const.tile([1, 512], BF16)
        nc.vector.memset(zero_row[:], 0.0)
        # wmask[k, q] = 0 where q >= k else MASK_NEG (strict lower triangle
        # of keys over queries within the diagonal 128x128 block)
        wmask = const.tile([128, 128], BF16)
        nc.gpsimd.memset(wmask[:], 0.0)
        nc.gpsimd.affine_select(
            out=wmask[:],
            in_=wmask[:],
            compare_op=mybir.AluOpType.is_ge,
            fill=MASK_NEG,
            base=0,
            pattern=[[1, 128]],
            channel_multiplier=-1,
        )

        # ---- persistent activations ----
        # q8/k8: partition = (h%2)*64 + dh, free dims (pair, drslot, t).
        # drslot 1 is a constant zero operand: DoubleRow needs a [p, 2, n]
        # shape but the contraction is only 64 deep, so the second slot
        # multiplies zeros (and must be zeroed -- fp8 garbage can be NaN).
        q8 = persist.tile([128, 2, 2, T], F8)
        k8 = persist.tile([128, 2, 2, T], F8)
        nc.gpsimd.memset(q8[:, :, 1, :], 0.0)
        nc.gpsimd.memset(k8[:, :, 1, :], 0.0)
        # v: partition = key%128, free (ktile, head, dh+ones)
        v_sb = persist.tile([128, NKT, HPG, DH + 1], BF16)
        nc.vector.memset(v_sb[:, :, :, DH:DH + 1], 1.0)

        def proj_units(n):
            """Chunk-n projection emission as self-contained closures (one
            complete PSUM accumulation group each) so they can be spread
            across the previous chunk's exp-bound attention phase."""
            if n >= NCHUNK:
                return []
            cols = slice(n * CHUNK, (n + 1) * CHUNK)
            xs = {}

            def get_x():
                if "x" not in xs:
                    xs["x"] = load_x(n)
                    prefetch_x(n + 1)
                return xs["x"]

            units = []

            def qk_unit(w_pair, b_sb, dst, mt, tag):
                def run():
                    xthi, xtlo = get_x()
                    terms = ((xthi, 0), (xthi, 1), (xtlo, 0))
                    ps = ps_big.tile([128, 2, CHUNK], F32, tag="ps",
                                     name=f"p{tag}{n}_{mt}")
                    i, nmm = 0, len(terms) * KO2
                    for xt, wi in terms:
                        for ko in range(KO2):
                            nc.tensor.matmul(
                                ps[:, 0, :],
                                lhsT=w_pair[wi][:, :, ko, mt * 128:(mt + 1) * 128],
                                rhs=xt[:, :, ko, :],
                                start=(i == 0),
                                stop=(i == nmm - 1),
                                perf_mode=DR,
                            )
                            i += 1
                    nc.vector.tensor_scalar(
                        dst[:, mt, 0, cols], ps[:, 0, :],
                        SQ / (SX * SW), b_sb[:, mt:mt + 1],
                        op0=mybir.AluOpType.mult, op1=mybir.AluOpType.add,
                    )
                return run

            def v_unit(tt):
                def run():
                    xthi, xtlo = get_x()
                    terms = ((xthi, 0), (xthi, 1), (xtlo, 0))
                    kt = 4 * n + tt
                    ps = ps_big.tile([128, 2, CHUNK], F32, tag="ps",
                                     name=f"pv{n}_{tt}")
                    i, nmm = 0, len(terms) * KO2
                    for xt, wi in terms:
                        for ko in range(KO2):
                            nc.tensor.matmul(
                                ps[:, 0, 0:DG],
                                lhsT=xt[:, :, ko, tt * 128:(tt + 1) * 128],
                                rhs=wv_sb[wi][:, :, ko, :],
                                start=(i == 0),
                                stop=(i == nmm - 1),
                                perf_mode=DR,
                            )
                            i += 1
                    nc.vector.tensor_scalar_mul(
                        v_sb[:, kt, :, 0:DH], ps[:, 0, 0:DG], 1.0 / (SX * SW)
                    )
                return run

            for mt in range(2):
                units.append(qk_unit(wq_sb, bq_sb, q8, mt, "q"))
                units.append(qk_unit(wk_sb, bk_sb, k8, mt, "k"))
            for tt in range(4):
                units.append(v_unit(tt))
            return units

        DELAY = 3

        carry_out = []
        pre_scores = {}

        def emit_scores(sn, p, j):
            """Score matmuls + causal mask + exp for (chunk sn, pair p,
            key tile j); returns the bf16 e tile."""
            qlo = max(0, 128 * (j - 4 * sn))
            diag = j >= 4 * sn
            pss = ps_big.tile([128, 2, CHUNK], F32, tag="ps",
                              name=f"ss{sn}_{p}_{j}")
            for h01 in range(2):
                rows = slice(64 * h01, 64 * h01 + 64)
                nc.tensor.matmul(
                    pss[:, h01, qlo:],
                    lhsT=k8[rows, p, :, j * 128:(j + 1) * 128],
                    rhs=q8[rows, p, :,
                           sn * CHUNK + qlo:(sn + 1) * CHUNK],
                    start=True,
                    stop=not diag,
                    perf_mode=DR,
                )
            if diag:
                for h01 in range(2):
                    nc.tensor.matmul(
                        pss[:, h01, qlo:qlo + 128],
                        lhsT=ident[:],
                        rhs=wmask[:],
                        start=False,
                        stop=True,
                    )
            e = epool.tile([128, 2, CHUNK], BF16, tag="e")
            nc.scalar.activation(
                e[:, :, qlo:], pss[:, :, qlo:],
                mybir.ActivationFunctionType.Exp,
                scale=1.0 / (8.0 * SQ * SQ),
            )
            return e

        def attention(n, feed):
            """Scores + attn@v for chunk n (two head-pair passes). attn@v
            runs DELAY score-tiles behind the exp producing its input, and
            units from `feed` (next chunk's projection) are spread evenly
            over the score tiles to fill the PE while ACT works through
            the exps."""
            njt = 4 * (n + 1)
            steps_total = 2 * njt
            step_no = [0]
            nfeed = len(feed)
            consumed = [0]
            carried = list(carry_out)
            carry_out.clear()
            y_sb = ypool.tile([128, 4, HPG, DH], BF16, tag="y", name=f"y{n}")
            yt_sb = ytpool.tile([128, 2, CHUNK], BF16, tag="yt", name=f"yt{n}")
            pending_av = []
            pending_out = []

            def normalize(p, qt, psyA, psyB):
                rec = small.tile([128, 2], F32, tag="rec", name=f"rc{n}_{p}_{qt}")
                lv = small.tile([128, 2], F32, tag="lv", name=f"lv{n}_{p}_{qt}")
                nc.vector.tensor_copy(lv[:, 0:1], psyA[:, qt, DH:DH + 1])
                nc.vector.tensor_copy(lv[:, 1:2], psyB[:, qt, DH:DH + 1])
                nc.vector.reciprocal(rec[:], lv[:])
                for h01, psy in ((0, psyA), (1, psyB)):
                    nc.vector.tensor_scalar_mul(
                        y_sb[:, qt, 2 * p + h01, :], psy[:, qt, 0:DH],
                        rec[:, h01:h01 + 1],
                    )

            def transpose_y(qt):
                # y[q, ydim] -> yT[ydim, t] via the PE transpose datapath;
                # output lands bf16 in a bitcast corner of an outproj-pool
                # tile (keeps the scores-pool rotation free of chunk-tail
                # eviction dependencies)
                tp = ps_big.tile([128, 2, CHUNK], F32, tag="ps", name=f"tp{n}_{qt}")
                tpb = tp[:].bitcast(BF16)
                for mt in range(2):
                    nc.tensor.matmul(
                        tpb[:, mt, 0:128],
                        lhsT=y_sb[:, qt, 2 * mt:2 * mt + 2, :],
                        rhs=ident[:],
                        is_transpose=True,
                    )
                nc.vector.tensor_copy(
                    yt_sb[:, :, qt * 128:(qt + 1) * 128], tpb[:, :, 0:128]
                )

            def outproj(qt):
                t_tile = 4 * n + qt
                o_sb = opool.tile([128, C], BF16, tag="o", name=f"o{n}_{qt}")
                for nh in range(2):
                    ps = ps_o.tile([128, 512], F32, tag="o", name=f"po{n}_{qt}_{nh}")
                    for mt in range(2):
                        nc.tensor.matmul(
                            ps[:],
                            lhsT=yt_sb[:, mt, qt * 128:(qt + 1) * 128],
                            rhs=wpt_sb[:, mt, nh * 512:(nh + 1) * 512],
                            start=(mt == 0),
                            stop=(mt == 1),
                        )
                    nc.vector.tensor_copy(o_sb[:, nh * 512:(nh + 1) * 512], ps[:])
                nc.sync.dma_start(out[t_tile], o_sb[:])

            def attnv(p, j, e, psyA, psyB):
                def run():
                    qlo = max(0, 128 * (j - 4 * n))
                    for qt in range(qlo // 128, 4):
                        for h01, psy in ((0, psyA), (1, psyB)):
                            nc.tensor.matmul(
                                psy[:, qt, 0:DH + 1],
                                lhsT=e[:, h01, qt * 128:(qt + 1) * 128],
                                rhs=v_sb[:, j, 2 * p + h01, :],
                                start=False,
                                stop=(j == 4 * n + qt),
                                skip_group_check=True,
                            )
                    if j >= 4 * n:
                        qt_done = j - 4 * n
                        normalize(p, qt_done, psyA, psyB)
                        if p == 1:
                            transpose_y(qt_done)
                            if n < NCHUNK - 1:
                                carry_out.append(
                                    lambda qt=qt_done: outproj(qt))
                            else:
                                pending_out.append(qt_done)
                                if len(pending_out) > 1:
                                    outproj(pending_out.pop(0))
                return run

            def step():
                if pending_av and len(pending_av) > DELAY:
                    pending_av.pop(0)()
                step_no[0] += 1
                if carried and step_no[0] % 2 == 0:
                    carried.pop(0)()
                # spread feed units so they finish ~7/8 through the chunk
                if feed:
                    target = min(len(feed) + consumed[0],
                                 (nfeed * 8 * step_no[0])
                                 // (7 * steps_total) + 1)
                    while consumed[0] < target and feed:
                        feed.pop(0)()
                        consumed[0] += 1

            for p in range(2):
                psyA = ps_y.tile([128, 4, 128], F32, tag="psy", name=f"pyA{n}_{p}")
                psyB = ps_y.tile([128, 4, 128], F32, tag="psy", name=f"pyB{n}_{p}")
                # one start per PSUM bank: a K=1 matmul zeroes the whole
                # bank so every attn@v slot can accumulate with start=False
                # (multiple start groups in one 2KB region clobber siblings)
                for psy in (psyA, psyB):
                    nc.tensor.matmul(
                        psy[:, :, :].rearrange("p a b -> p (a b)"),
                        lhsT=ones_row[:],
                        rhs=zero_row[:],
                        start=True,
                        stop=True,
                        skip_group_check=True,
                    )
                for j in range(njt):
                    e = pre_scores.pop((n, p, j), None)
                    if e is None:
                        e = emit_scores(n, p, j)
                    if p == 1 and j == njt - 1 and n + 1 < NCHUNK:
                        # pre-emit the next chunk's first two score tiles so
                        # ACT streams straight through the chunk boundary
                        for jj in range(2):
                            pre_scores[(n + 1, 0, jj)] = emit_scores(
                                n + 1, 0, jj)
                    pending_av.append(attnv(p, j, e, psyA, psyB))
                    step()
            while pending_av:
                pending_av.pop(0)()
                while feed:
                    feed.pop(0)()
            for fn in carried:
                fn()
            if n < NCHUNK - 1:
                for qt in pending_out:
                    carry_out.append(lambda qt=qt: outproj(qt))
            else:
                for qt in pending_out:
                    outproj(qt)

        for u in proj_units(0):
            u()
        for n in range(NCHUNK):
            feed = proj_units(n + 1)
            attention(n, feed)
            for u in feed:
                u()

    return nc


_NC_CACHE = None


def _split8(a, s):
    hi = np.asarray(a * s, NPF8)
    lo = np.asarray(a * s - hi.astype(np.float32), NPF8)
    return hi, lo


def kernel(**inputs) -> np.ndarray:
    global _NC_CACHE
    x = np.asarray(inputs["x"], np.float32)
    Wq = np.asarray(inputs["Wq"], np.float32)
    Wk = np.asarray(inputs["Wk"], np.float32)
    Wv = np.asarray(inputs["Wv"], np.float32)
    Wp = np.asarray(inputs["Wp"], np.float32)
    bq = np.asarray(inputs["bq"], np.float32)
    bk = np.asarray(inputs["bk"], np.float32)
    bv = np.asarray(inputs["bv"], np.float32)
    bp = np.asarray(inputs["bp"], np.float32)

    if _NC_CACHE is None:
        _NC_CACHE = build_kernel()
    nc = _NC_CACHE

    def pack_w(Wl):
        # Wl: [256 out, 1024 in] slice -> lhsT [c, m] -> [p, slot, ko, m]
        wt = Wl.T                                          # [1024 c, 256 m]
        wt = wt.reshape(KO2, 2, 128, 256).transpose(2, 1, 0, 3)
        hi, lo = _split8(np.ascontiguousarray(wt), SW)
        return np.ascontiguousarray(hi), np.ascontiguousarray(lo)

    in_maps = []
    for c in range(NCORES):
        b, g = divmod(c, GROUPS)
        rows = slice(g * DG, (g + 1) * DG)
        xt = x[b].T.reshape(KO2, 2, 128, T).transpose(2, 1, 0, 3)  # [p,slot,ko,t]
        xt = xt.reshape(128, 2, KO2, NCHUNK, CHUNK).transpose(3, 0, 1, 2, 4)
        xhi, xlo = _split8(np.ascontiguousarray(xt), SX)

        wq_hi, wq_lo = pack_w(Wq[rows])
        wk_hi, wk_lo = pack_w(Wk[rows])
        wv_hi, wv_lo = pack_w(Wv[rows])
        wpt_l = np.ascontiguousarray(
            Wp[:, rows].T.reshape(2, 128, C).transpose(1, 0, 2)
        ).astype(NPBF16)

        bq4 = np.ascontiguousarray((bq[rows] * SQ).reshape(2, 128).T)
        bk4 = np.ascontiguousarray((bk[rows] * SQ).reshape(2, 128).T)

        in_maps.append({
            "xh": np.ascontiguousarray(xhi),
            "xl": np.ascontiguousarray(xlo),
            "wqh": wq_hi, "wql": wq_lo,
            "wkh": wk_hi, "wkl": wk_lo,
            "wvh": wv_hi, "wvl": wv_lo,
            "wpt": wpt_l,
            "bq4": bq4, "bk4": bk4,
        })

    res = run_bass_kernel_spmd(nc, in_maps, core_ids=list(range(NCORES)))

    result = np.zeros((B, T, C), np.float32)
    for c in range(NCORES):
        b = c // GROUPS
        o = np.asarray(res.results[c]["out"]).astype(np.float32)
        result[b] += o.reshape(T, C)
    result += (bv @ Wp.T + bp)[None, None, :]
    return result


# revision 48
# speedup vs baseline: 1.0027x; 1.0027x over previous
"""Causal self-attention on 8 trn2 NeuronCores.

Sharding: core c -> (batch b = c // 4, head-group g = c % 4). Each core
computes 4 of the 16 heads for one batch element plus its slice of the
output projection; the host sums the 4 partial projections per batch and
adds the constant (bv @ Wp.T + bp) term exactly.

Kernel structure (per core), streamed over 4 query chunks of 512:
  - Q/K/V projections as fp8e4 DoubleRow matmuls with hi+lo residual
    splits of both x and W (3 accumulation terms; quantization error
    ~0.05%), contraction 256/step.
  - Scores s = k.T q in fp8e4 DoubleRow ([keys, queries] orientation,
    dh packed 32x2), causal mask added in PSUM via an identity matmul,
    exp on the Activation engine straight out of PSUM into bf16 SBUF.
  - attn@v flipped: e is the stationary operand, v (with a trailing
    ones column that accumulates the softmax denominator l) is moving;
    PSUM rows are queries so 1/l is a per-partition scalar folded into
    the eviction tensor_scalar op.
  - y transposed via the DMA xbar (16x128 tiles) into [ydim, t] layout,
    then the output projection in bf16; out partials stored bf16.
"""

import numpy as np
import ml_dtypes

import concourse.bass as bass
import concourse.mybir as mybir
import concourse.tile as tile
from concourse.bass_utils import run_bass_kernel_spmd

B = 2
T = 2048
C = 1024
H = 16
DH = 64
NCORES = 8
GROUPS = 4            # head groups (tensor parallel)
HPG = H // GROUPS     # heads per group = 4
DG = HPG * DH         # head-group width = 256
CHUNK = 512           # query-chunk size
NCHUNK = T // CHUNK   # 4
KO2 = C // 256        # DoubleRow contraction steps for the projections
NKT = T // 128        # key tiles
F32 = mybir.dt.float32
F32R = mybir.dt.float32r
BF16 = mybir.dt.bfloat16
F8 = mybir.dt.float8e4
NPF8 = ml_dtypes.float8_e4m3
NPBF16 = ml_dtypes.bfloat16
MASK_NEG = -1e30

SX = 16.0             # x fp8 scale
SW = 256.0            # weight fp8 scale
SQ = 4.0              # q/k fp8 store scale
DR = mybir.MatmulPerfMode.DoubleRow


def _patch_tile_drain():
    """This walrus build lowers Drain/NOP to a CTRL with a single sync-wait
    slot; TileContext's kernel-tail drain accumulates one wait per live
    semaphore and fails codegen. Split the waits across single-wait NOPs."""
    import bass_rust
    from concourse.tile import TileContext

    def _drain_and_barrier_split(self, tick_clock, wait_clock):
        probe = self.nc.sync.nop()
        wait_clock.add_sem_waits(
            probe.ins, tile.ScopedClock({None: tick_clock.global_clock})
        )
        waits = list(probe.ins.sync_info.on_wait or [])
        probe.ins.sync_info.on_wait = []
        engines = [self.nc.sync, self.nc.tensor, self.nc.vector,
                   self.nc.scalar, self.nc.gpsimd]
        for i, w in enumerate(waits):
            n = engines[i % len(engines)].nop()
            if n.ins.sync_info is None:
                n.ins.sync_info = bass_rust.SyncInfo(on_wait=[w], on_update=[])
            else:
                n.ins.sync_info.on_wait = [w]
        self.nc.sync.drain()
        self.nc.all_engine_barrier()
        assert self.sems is not None
        popped = self.nc._tile_sem_poison_stack.pop()
        assert popped is self._sem_poison
        self.nc.clear_and_free_semaphores(list(self.sems.allocated().values()))
        self.nc.all_engine_barrier()

    TileContext._drain_and_barrier = _drain_and_barrier_split

    import json as _json

    import concourse.bass2jax as bass2jax
    import concourse.bass_utils as bass_utils

    if getattr(bass_utils.compile_bir_kernel, "_wait_split", False):
        return

    _orig_compile = bass_utils.compile_bir_kernel

    def _split_multi_waits(bir_json):
        m = _json.loads(bir_json)
        counter = 0
        changed = False
        for fn in m["functions"]:
            for blk in fn["blocks"]:
                new_insts = []
                for inst in blk["instructions"]:
                    si = inst.get("sync_info")
                    waits = (si or {}).get("on_wait") or []
                    sem_waits = [w for w in waits if w.get("sync_type") == "semaphore"]
                    if len(waits) > 1 and len(sem_waits) == len(waits):
                        changed = True
                        for w in waits[:-1]:
                            counter += 1
                            new_insts.append({
                                "name": f"I-wsplit{counter}",
                                "opcode": "NoOp",
                                "engine": inst["engine"],
                                "ins": [],
                                "outs": [],
                                "sync_info": {"on_wait": [w], "on_update": []},
                            })
                        si["on_wait"] = [waits[-1]]
                    new_insts.append(inst)
                blk["instructions"] = new_insts
        if not changed:
            return bir_json
        return _json.dumps(m).encode()

    def _compile_bir_kernel_split(bir_json, tmpdir, neff_name="file.neff"):
        return _orig_compile(_split_multi_waits(bir_json), tmpdir, neff_name=neff_name)

    _compile_bir_kernel_split._wait_split = True
    bass_utils.compile_bir_kernel = _compile_bir_kernel_split
    bass2jax.compile_bir_kernel = _compile_bir_kernel_split


def build_kernel():
    _patch_tile_drain()
    nc = bass.Bass(target_bir_lowering=False, trn_type="TRN2")

    # hi/lo fp8 operand pairs; layouts are DoubleRow-packed on the host:
    # contraction index c = ko*256 + slot*128 + p.
    xh = nc.dram_tensor("xh", [NCHUNK, 128, 2, KO2, CHUNK], F8, kind="ExternalInput")
    xl = nc.dram_tensor("xl", [NCHUNK, 128, 2, KO2, CHUNK], F8, kind="ExternalInput")
    wqh = nc.dram_tensor("wqh", [128, 2, KO2, DG], F8, kind="ExternalInput")
    wql = nc.dram_tensor("wql", [128, 2, KO2, DG], F8, kind="ExternalInput")
    wkh = nc.dram_tensor("wkh", [128, 2, KO2, DG], F8, kind="ExternalInput")
    wkl = nc.dram_tensor("wkl", [128, 2, KO2, DG], F8, kind="ExternalInput")
    wvh = nc.dram_tensor("wvh", [128, 2, KO2, DG], F8, kind="ExternalInput")
    wvl = nc.dram_tensor("wvl", [128, 2, KO2, DG], F8, kind="ExternalInput")
    wpt = nc.dram_tensor("wpt", [128, 2, C], BF16, kind="ExternalInput")
    bq4 = nc.dram_tensor("bq4", [128, 2], F32, kind="ExternalInput")
    bk4 = nc.dram_tensor("bk4", [128, 2], F32, kind="ExternalInput")
    out = nc.dram_tensor("out", [NKT, 128, C], BF16, kind="ExternalOutput")

    from contextlib import ExitStack

    with tile.TileContext(nc) as tc, ExitStack() as ctx:
        from concourse.masks import make_identity

        const = ctx.enter_context(tc.tile_pool(name="const", bufs=1))
        xpool = ctx.enter_context(tc.tile_pool(name="xp", bufs=4))
        persist = ctx.enter_context(tc.tile_pool(name="persist", bufs=1))
        epool = ctx.enter_context(tc.tile_pool(name="ep", bufs=5))
        ypool = ctx.enter_context(tc.tile_pool(name="yp", bufs=2))
        ytpool = ctx.enter_context(tc.tile_pool(name="ytp", bufs=4))
        opool = ctx.enter_context(tc.tile_pool(name="op", bufs=3))
        small = ctx.enter_context(tc.tile_pool(name="sm", bufs=6))
        ps_big = ctx.enter_context(tc.tile_pool(name="psb", bufs=2, space="PSUM"))
        ps_y = ctx.enter_context(tc.tile_pool(name="psy", bufs=2, space="PSUM"))
        ps_o = ctx.enter_context(tc.tile_pool(name="pso", bufs=2, space="PSUM"))

        _x_tiles = {}

        def prefetch_x(n):
            if n not in _x_tiles and n < NCHUNK:
                th = xpool.tile([128, 2, KO2, CHUNK], F8, tag="x", name=f"xh{n}")
                nc.sync.dma_start(th[:], xh[n])
                tl = xpool.tile([128, 2, KO2, CHUNK], F8, tag="x", name=f"xl{n}")
                nc.sync.dma_start(tl[:], xl[n])
                _x_tiles[n] = (th, tl)

        def load_x(n):
            prefetch_x(n)
            return _x_tiles.pop(n)

        # ---- constants ----  (x chunk 0 is prefetched right after the wq
        # pair so the first projection matmul can start ~2.5us in)
        wq_sb, wk_sb, wv_sb = [], [], []
        _w_srcs = ((wq_sb, wqh, wql), (wk_sb, wkh, wkl), (wv_sb, wvh, wvl))
        _w_tiles = []
        for wn, (dst, hi, lo) in enumerate(_w_srcs):
            for hl, w_dram in enumerate((hi, lo)):
                t = const.tile([128, 2, KO2, DG], F8, name=f"w{wn}_{hl}")
                _w_tiles.append((t, w_dram))
                dst.append(t)
        _order = [0, 2, 1, 3, 4, 5]        # wq-hi, wk-hi, wq-lo, wk-lo, wv
        nc.sync.dma_start(_w_tiles[0][0][:], _w_tiles[0][1][:])   # wq hi
        nc.sync.dma_start(_w_tiles[2][0][:], _w_tiles[2][1][:])   # wk hi
        prefetch_x(0)
        for wi in (1, 3, 4, 5):
            t, w_dram = _w_tiles[wi]
            nc.sync.dma_start(t[:], w_dram[:])
        bq_sb = const.tile([128, 2], F32)
        nc.sync.dma_start(bq_sb[:], bq4[:])
        bk_sb = const.tile([128, 2], F32)
        nc.sync.dma_start(bk_sb[:], bk4[:])
        wpt_sb = const.tile([128, 2, C], BF16)
        nc.sync.dma_start(wpt_sb[:], wpt[:])

        ident = const.tile([128, 128], BF16)
        make_identity(nc, ident)
        ones_row = const.tile([1, 128], BF16)
        nc.vector.memset(ones_row[:], 1.0)
        zero_row = const.tile([1, 512], BF16)
        nc.vector.memset(zero_row[:], 0.0)
        # wmask[k, q] = 0 where q >= k else MASK_NEG (strict lower triangle
        # of keys over queries within the diagonal 128x128 block)
        wmask = const.tile([128, 128], BF16)
        nc.gpsimd.memset(wmask[:], 0.0)
        nc.gpsimd.affine_select(
            out=wmask[:],
            in_=wmask[:],
            compare_op=mybir.AluOpType.is_ge,
            fill=MASK_NEG,
            base=0,
            pattern=[[1, 128]],
            channel_multiplier=-1,
        )

        # ---- persistent activations ----
        # q8/k8: partition = (h%2)*64 + dh, free dims (pair, drslot, t).
        # drslot 1 is a constant zero operand: DoubleRow needs a [p, 2, n]
        # shape but the contraction is only 64 deep, so the second slot
        # multiplies zeros (and must be zeroed -- fp8 garbage can be NaN).
        q8 = persist.tile([128, 2, 2, T], F8)
        k8 = persist.tile([128, 2, 2, T], F8)
        nc.gpsimd.memset(q8[:, :, 1, :], 0.0)
        nc.gpsimd.memset(k8[:, :, 1, :], 0.0)
        # v: partition = key%128, free (ktile, head, dh+ones)
        v_sb = persist.tile([128, NKT, HPG, DH + 1], BF16)
        nc.vector.memset(v_sb[:, :, :, DH:DH + 1], 1.0)

        def proj_units(n):
            """Chunk-n projection emission as self-contained closures (one
            complete PSUM accumulation group each) so they can be spread
            across the previous chunk's exp-bound attention phase."""
            if n >= NCHUNK:
                return []
            cols = slice(n * CHUNK, (n + 1) * CHUNK)
            xs = {}

            def get_x():
                if "x" not in xs:
                    xs["x"] = load_x(n)
                    prefetch_x(n + 1)
                return xs["x"]

            units = []

            def qk_unit(w_pair, b_sb, dst, mt, tag):
                def run():
                    xthi, xtlo = get_x()
                    terms = ((xthi, 0), (xthi, 1), (xtlo, 0))
                    ps = ps_big.tile([128, 2, CHUNK], F32, tag="ps",
                                     name=f"p{tag}{n}_{mt}")
                    i, nmm = 0, len(terms) * KO2
                    for xt, wi in terms:
                        for ko in range(KO2):
                            nc.tensor.matmul(
                                ps[:, 0, :],
                                lhsT=w_pair[wi][:, :, ko, mt * 128:(mt + 1) * 128],
                                rhs=xt[:, :, ko, :],
                                start=(i == 0),
                                stop=(i == nmm - 1),
                                perf_mode=DR,
                            )
                            i += 1
                    nc.vector.tensor_scalar(
                        dst[:, mt, 0, cols], ps[:, 0, :],
                        SQ / (SX * SW), b_sb[:, mt:mt + 1],
                        op0=mybir.AluOpType.mult, op1=mybir.AluOpType.add,
                    )
                return run

            def v_unit(tt):
                def run():
                    xthi, xtlo = get_x()
                    terms = ((xthi, 0), (xthi, 1), (xtlo, 0))
                    kt = 4 * n + tt
                    ps = ps_big.tile([128, 2, CHUNK], F32, tag="ps",
                                     name=f"pv{n}_{tt}")
                    i, nmm = 0, len(terms) * KO2
                    for xt, wi in terms:
                        for ko in range(KO2):
                            nc.tensor.matmul(
                                ps[:, 0, 0:DG],
                                lhsT=xt[:, :, ko, tt * 128:(tt + 1) * 128],
                                rhs=wv_sb[wi][:, :, ko, :],
                                start=(i == 0),
                                stop=(i == nmm - 1),
                                perf_mode=DR,
                            )
                            i += 1
                    nc.vector.tensor_scalar_mul(
                        v_sb[:, kt, :, 0:DH], ps[:, 0, 0:DG], 1.0 / (SX * SW)
                    )
                return run

            for mt in range(2):
                units.append(qk_unit(wq_sb, bq_sb, q8, mt, "q"))
                units.append(qk_unit(wk_sb, bk_sb, k8, mt, "k"))
            for tt in range(4):
                units.append(v_unit(tt))
            return units

        DELAY = 3

        carry_out = []
        pre_scores = {}

        def emit_scores(sn, p, j):
            """Score matmuls + causal mask + exp for (chunk sn, pair p,
            key tile j); returns the bf16 e tile."""
            qlo = max(0, 128 * (j - 4 * sn))
            diag = j >= 4 * sn
            pss = ps_big.tile([128, 2, CHUNK], F32, tag="ps",
                              name=f"ss{sn}_{p}_{j}")
            for h01 in range(2):
                rows = slice(64 * h01, 64 * h01 + 64)
                nc.tensor.matmul(
                    pss[:, h01, qlo:],
                    lhsT=k8[rows, p, :, j * 128:(j + 1) * 128],
                    rhs=q8[rows, p, :,
                           sn * CHUNK + qlo:(sn + 1) * CHUNK],
                    start=True,
                    stop=not diag,
                    perf_mode=DR,
                )
            if diag:
                for h01 in range(2):
                    nc.tensor.matmul(
                        pss[:, h01, qlo:qlo + 128],
                        lhsT=ident[:],
                        rhs=wmask[:],
                        start=False,
                        stop=True,
                    )
            e = epool.tile([128, 2, CHUNK], BF16, tag="e")
            nc.scalar.activation(
                e[:, :, qlo:], pss[:, :, qlo:],
                mybir.ActivationFunctionType.Exp,
                scale=1.0 / (8.0 * SQ * SQ),
            )
            return e

        def attention(n, feed):
            """Scores + attn@v for chunk n (two head-pair passes). attn@v
            runs DELAY score-tiles behind the exp producing its input, and
            units from `feed` (next chunk's projection) are spread evenly
            over the score tiles to fill the PE while ACT works through
            the exps."""
            njt = 4 * (n + 1)
            steps_total = 2 * njt
            step_no = [0]
            nfeed = len(feed)
            consumed = [0]
            carried = list(carry_out)
            carry_out.clear()
            y_sb = ypool.tile([128, 4, HPG, DH], BF16, tag="y", name=f"y{n}")
            yt_sb = ytpool.tile([128, 2, CHUNK], BF16, tag="yt", name=f"yt{n}")
            pending_av = []
            pending_out = []

            def normalize(p, qt, psyA, psyB):
                rec = small.tile([128, 2], F32, tag="rec", name=f"rc{n}_{p}_{qt}")
                lv = small.tile([128, 2], F32, tag="lv", name=f"lv{n}_{p}_{qt}")
                nc.vector.tensor_copy(lv[:, 0:1], psyA[:, qt, DH:DH + 1])
                nc.vector.tensor_copy(lv[:, 1:2], psyB[:, qt, DH:DH + 1])
                nc.vector.reciprocal(rec[:], lv[:])
                for h01, psy in ((0, psyA), (1, psyB)):
                    nc.vector.tensor_scalar_mul(
                        y_sb[:, qt, 2 * p + h01, :], psy[:, qt, 0:DH],
                        rec[:, h01:h01 + 1],
                    )

            def transpose_y(qt):
                # y[q, ydim] -> yT[ydim, t] via the PE transpose datapath;
                # output lands bf16 in a bitcast corner of an outproj-pool
                # tile (keeps the scores-pool rotation free of chunk-tail
                # eviction dependencies)
                tp = ps_big.tile([128, 2, CHUNK], F32, tag="ps", name=f"tp{n}_{qt}")
                tpb = tp[:].bitcast(BF16)
                for mt in range(2):
                    nc.tensor.matmul(
                        tpb[:, mt, 0:128],
                        lhsT=y_sb[:, qt, 2 * mt:2 * mt + 2, :],
                        rhs=ident[:],
                        is_transpose=True,
                    )
                nc.vector.tensor_copy(
                    yt_sb[:, :, qt * 128:(qt + 1) * 128], tpb[:, :, 0:128]
                )

            def outproj(qt):
                t_tile = 4 * n + qt
                o_sb = opool.tile([128, C], BF16, tag="o", name=f"o{n}_{qt}")
                for nh in range(2):
                    ps = ps_o.tile([128, 512], F32, tag="o", name=f"po{n}_{qt}_{nh}")
                    for mt in range(2):
                        nc.tensor.matmul(
                            ps[:],
                            lhsT=yt_sb[:, mt, qt * 128:(qt + 1) * 128],
                            rhs=wpt_sb[:, mt, nh * 512:(nh + 1) * 512],
                            start=(mt == 0),
                            stop=(mt == 1),
                        )
                    nc.vector.tensor_copy(o_sb[:, nh * 512:(nh + 1) * 512], ps[:])
                    nc.sync.dma_start(
                        out[t_tile][:, nh * 512:(nh + 1) * 512],
                        o_sb[:, nh * 512:(nh + 1) * 512],
                    )

            def attnv(p, j, e, psyA, psyB):
                def run():
                    qlo = max(0, 128 * (j - 4 * n))
                    for qt in range(qlo // 128, 4):
                        for h01, psy in ((0, psyA), (1, psyB)):
                            nc.tensor.matmul(
                                psy[:, qt, 0:DH + 1],
                                lhsT=e[:, h01, qt * 128:(qt + 1) * 128],
                                rhs=v_sb[:, j, 2 * p + h01, :],
                                start=False,
                                stop=(j == 4 * n + qt),
                                skip_group_check=True,
                            )
                    if j >= 4 * n:
                        qt_done = j - 4 * n
                        normalize(p, qt_done, psyA, psyB)
                        if p == 1:
                            transpose_y(qt_done)
                            if n < NCHUNK - 1:
                                carry_out.append(
                                    lambda qt=qt_done: outproj(qt))
                            else:
                                pending_out.append(qt_done)
                                if len(pending_out) > 1:
                                    outproj(pending_out.pop(0))
                return run

            def step():
                if pending_av and len(pending_av) > DELAY:
                    pending_av.pop(0)()
                step_no[0] += 1
                if carried and step_no[0] % 2 == 0:
                    carried.pop(0)()
                # spread feed units so they finish ~7/8 through the chunk
                if feed:
                    target = min(len(feed) + consumed[0],
                                 (nfeed * 8 * step_no[0])
                                 // (7 * steps_total) + 1)
                    while consumed[0] < target and feed:
                        feed.pop(0)()
                        consumed[0] += 1

            for p in range(2):
                psyA = ps_y.tile([128, 4, 128], F32, tag="psy", name=f"pyA{n}_{p}")
                psyB = ps_y.tile([128, 4, 128], F32, tag="psy", name=f"pyB{n}_{p}")
                # one start per PSUM bank: a K=1 matmul zeroes the whole
                # bank so every attn@v slot can accumulate with start=False
                # (multiple start groups in one 2KB region clobber siblings)
                for psy in (psyA, psyB):
                    nc.tensor.matmul(
                        psy[:, :, :].rearrange("p a b -> p (a b)"),
                        lhsT=ones_row[:],
                        rhs=zero_row[:],
                        start=True,
                        stop=True,
                        skip_group_check=True,
                    )
                for j in range(njt):
                    e = pre_scores.pop((n, p, j), None)
                    if e is None:
                        e = emit_scores(n, p, j)
                    if p == 1 and j == njt - 1 and n + 1 < NCHUNK:
                        # pre-emit the next chunk's first two score tiles so
                        # ACT streams straight through the chunk boundary
                        for jj in range(2):
                            pre_scores[(n + 1, 0, jj)] = emit_scores(
                                n + 1, 0, jj)
                    pending_av.append(attnv(p, j, e, psyA, psyB))
                    step()
            while pending_av:
                pending_av.pop(0)()
                while feed:
                    feed.pop(0)()
            for fn in carried:
                fn()
            if n < NCHUNK - 1:
                for qt in pending_out:
                    carry_out.append(lambda qt=qt: outproj(qt))
            else:
                for qt in pending_out:
                    outproj(qt)

        for u in proj_units(0):
            u()
        for n in range(NCHUNK):
            feed = proj_units(n + 1)
            attention(n, feed)
            for u in feed:
                u()

    return nc


_NC_CACHE = None


def _split8(a, s):
    hi = np.asarray(a * s, NPF8)
    lo = np.asarray(a * s - hi.astype(np.float32), NPF8)
    return hi, lo


def kernel(**inputs) -> np.ndarray:
    global _NC_CACHE
    x = np.asarray(inputs["x"], np.float32)
    Wq = np.asarray(inputs["Wq"], np.float32)
    Wk = np.asarray(inputs["Wk"], np.float32)
    Wv = np.asarray(inputs["Wv"], np.float32)
    Wp = np.asarray(inputs["Wp"], np.float32)
    bq = np.asarray(inputs["bq"], np.float32)
    bk = np.asarray(inputs["bk"], np.float32)
    bv = np.asarray(inputs["bv"], np.float32)
    bp = np.asarray(inputs["bp"], np.float32)

    if _NC_CACHE is None:
        _NC_CACHE = build_kernel()
    nc = _NC_CACHE

    def pack_w(Wl):
        # Wl: [256 out, 1024 in] slice -> lhsT [c, m] -> [p, slot, ko, m]
        wt = Wl.T                                          # [1024 c, 256 m]
        wt = wt.reshape(KO2, 2, 128, 256).transpose(2, 1, 0, 3)
        hi, lo = _split8(np.ascontiguousarray(wt), SW)
        return np.ascontiguousarray(hi), np.ascontiguousarray(lo)

    in_maps = []
    for c in range(NCORES):
        b, g = divmod(c, GROUPS)
        rows = slice(g * DG, (g + 1) * DG)
        xt = x[b].T.reshape(KO2, 2, 128, T).transpose(2, 1, 0, 3)  # [p,slot,ko,t]
        xt = xt.reshape(128, 2, KO2, NCHUNK, CHUNK).transpose(3, 0, 1, 2, 4)
        xhi, xlo = _split8(np.ascontiguousarray(xt), SX)

        wq_hi, wq_lo = pack_w(Wq[rows])
        wk_hi, wk_lo = pack_w(Wk[rows])
        wv_hi, wv_lo = pack_w(Wv[rows])
        wpt_l = np.ascontiguousarray(
            Wp[:, rows].T.reshape(2, 128, C).transpose(1, 0, 2)
        ).astype(NPBF16)

        bq4 = np.ascontiguousarray((bq[rows] * SQ).reshape(2, 128).T)
        bk4 = np.ascontiguousarray((bk[rows] * SQ).reshape(2, 128).T)

        in_maps.append({
            "xh": np.ascontiguousarray(xhi),
            "xl": np.ascontiguousarray(xlo),
            "wqh": wq_hi, "wql": wq_lo,
            "wkh": wk_hi, "wkl": wk_lo,
            "wvh": wv_hi, "wvl": wv_lo,
            "wpt": wpt_l,
            "bq4": bq4, "bk4": bk4,
        })

    res = run_bass_kernel_spmd(nc, in_maps, core_ids=list(range(NCORES)))

    result = np.zeros((B, T, C), np.float32)
    for c in range(NCORES):
        b = c // GROUPS
        o = np.asarray(res.results[c]["out"]).astype(np.float32)
        result[b] += o.reshape(T, C)
    result += (bv @ Wp.T + bp)[None, None, :]
    return result


# revision 49
# speedup vs baseline: 1.0063x; 1.0035x over previous
"""Causal self-attention on 8 trn2 NeuronCores.

Sharding: core c -> (batch b = c // 4, head-group g = c % 4). Each core
computes 4 of the 16 heads for one batch element plus its slice of the
output projection; the host sums the 4 partial projections per batch and
adds the constant (bv @ Wp.T + bp) term exactly.

Kernel structure (per core), streamed over 4 query chunks of 512:
  - Q/K/V projections as fp8e4 DoubleRow matmuls with hi+lo residual
    splits of both x and W (3 accumulation terms; quantization error
    ~0.05%), contraction 256/step.
  - Scores s = k.T q in fp8e4 DoubleRow ([keys, queries] orientation,
    dh packed 32x2), causal mask added in PSUM via an identity matmul,
    exp on the Activation engine straight out of PSUM into bf16 SBUF.
  - attn@v flipped: e is the stationary operand, v (with a trailing
    ones column that accumulates the softmax denominator l) is moving;
    PSUM rows are queries so 1/l is a per-partition scalar folded into
    the eviction tensor_scalar op.
  - y transposed via the DMA xbar (16x128 tiles) into [ydim, t] layout,
    then the output projection in bf16; out partials stored bf16.
"""

import numpy as np
import ml_dtypes

import concourse.bass as bass
import concourse.mybir as mybir
import concourse.tile as tile
from concourse.bass_utils import run_bass_kernel_spmd

B = 2
T = 2048
C = 1024
H = 16
DH = 64
NCORES = 8
GROUPS = 4            # head groups (tensor parallel)
HPG = H // GROUPS     # heads per group = 4
DG = HPG * DH         # head-group width = 256
CHUNK = 512           # query-chunk size
NCHUNK = T // CHUNK   # 4
KO2 = C // 256        # DoubleRow contraction steps for the projections
NKT = T // 128        # key tiles
F32 = mybir.dt.float32
F32R = mybir.dt.float32r
BF16 = mybir.dt.bfloat16
F8 = mybir.dt.float8e4
NPF8 = ml_dtypes.float8_e4m3
NPBF16 = ml_dtypes.bfloat16
MASK_NEG = -1e30

SX = 16.0             # x fp8 scale
SW = 256.0            # weight fp8 scale
SQ = 4.0              # q/k fp8 store scale
DR = mybir.MatmulPerfMode.DoubleRow


def _patch_tile_drain():
    """This walrus build lowers Drain/NOP to a CTRL with a single sync-wait
    slot; TileContext's kernel-tail drain accumulates one wait per live
    semaphore and fails codegen. Split the waits across single-wait NOPs."""
    import bass_rust
    from concourse.tile import TileContext

    def _drain_and_barrier_split(self, tick_clock, wait_clock):
        probe = self.nc.sync.nop()
        wait_clock.add_sem_waits(
            probe.ins, tile.ScopedClock({None: tick_clock.global_clock})
        )
        waits = list(probe.ins.sync_info.on_wait or [])
        probe.ins.sync_info.on_wait = []
        engines = [self.nc.sync, self.nc.tensor, self.nc.vector,
                   self.nc.scalar, self.nc.gpsimd]
        for i, w in enumerate(waits):
            n = engines[i % len(engines)].nop()
            if n.ins.sync_info is None:
                n.ins.sync_info = bass_rust.SyncInfo(on_wait=[w], on_update=[])
            else:
                n.ins.sync_info.on_wait = [w]
        self.nc.sync.drain()
        self.nc.all_engine_barrier()
        assert self.sems is not None
        popped = self.nc._tile_sem_poison_stack.pop()
        assert popped is self._sem_poison
        self.nc.clear_and_free_semaphores(list(self.sems.allocated().values()))
        self.nc.all_engine_barrier()

    TileContext._drain_and_barrier = _drain_and_barrier_split

    import json as _json

    import concourse.bass2jax as bass2jax
    import concourse.bass_utils as bass_utils

    if getattr(bass_utils.compile_bir_kernel, "_wait_split", False):
        return

    _orig_compile = bass_utils.compile_bir_kernel

    def _split_multi_waits(bir_json):
        m = _json.loads(bir_json)
        counter = 0
        changed = False
        for fn in m["functions"]:
            for blk in fn["blocks"]:
                new_insts = []
                for inst in blk["instructions"]:
                    si = inst.get("sync_info")
                    waits = (si or {}).get("on_wait") or []
                    sem_waits = [w for w in waits if w.get("sync_type") == "semaphore"]
                    if len(waits) > 1 and len(sem_waits) == len(waits):
                        changed = True
                        for w in waits[:-1]:
                            counter += 1
                            new_insts.append({
                                "name": f"I-wsplit{counter}",
                                "opcode": "NoOp",
                                "engine": inst["engine"],
                                "ins": [],
                                "outs": [],
                                "sync_info": {"on_wait": [w], "on_update": []},
                            })
                        si["on_wait"] = [waits[-1]]
                    new_insts.append(inst)
                blk["instructions"] = new_insts
        if not changed:
            return bir_json
        return _json.dumps(m).encode()

    def _compile_bir_kernel_split(bir_json, tmpdir, neff_name="file.neff"):
        return _orig_compile(_split_multi_waits(bir_json), tmpdir, neff_name=neff_name)

    _compile_bir_kernel_split._wait_split = True
    bass_utils.compile_bir_kernel = _compile_bir_kernel_split
    bass2jax.compile_bir_kernel = _compile_bir_kernel_split


def build_kernel():
    _patch_tile_drain()
    nc = bass.Bass(target_bir_lowering=False, trn_type="TRN2")

    # hi/lo fp8 operand pairs; layouts are DoubleRow-packed on the host:
    # contraction index c = ko*256 + slot*128 + p.
    xh = nc.dram_tensor("xh", [NCHUNK, 128, 2, KO2, CHUNK], F8, kind="ExternalInput")
    xl = nc.dram_tensor("xl", [NCHUNK, 128, 2, KO2, CHUNK], F8, kind="ExternalInput")
    wqh = nc.dram_tensor("wqh", [128, 2, KO2, DG], F8, kind="ExternalInput")
    wql = nc.dram_tensor("wql", [128, 2, KO2, DG], F8, kind="ExternalInput")
    wkh = nc.dram_tensor("wkh", [128, 2, KO2, DG], F8, kind="ExternalInput")
    wkl = nc.dram_tensor("wkl", [128, 2, KO2, DG], F8, kind="ExternalInput")
    wvh = nc.dram_tensor("wvh", [128, 2, KO2, DG], F8, kind="ExternalInput")
    wvl = nc.dram_tensor("wvl", [128, 2, KO2, DG], F8, kind="ExternalInput")
    wpt = nc.dram_tensor("wpt", [128, 2, C], BF16, kind="ExternalInput")
    bq4 = nc.dram_tensor("bq4", [128, 2], F32, kind="ExternalInput")
    bk4 = nc.dram_tensor("bk4", [128, 2], F32, kind="ExternalInput")
    out = nc.dram_tensor("out", [NKT, 128, C], BF16, kind="ExternalOutput")

    from contextlib import ExitStack

    with tile.TileContext(nc) as tc, ExitStack() as ctx:
        from concourse.masks import make_identity

        const = ctx.enter_context(tc.tile_pool(name="const", bufs=1))
        xpool = ctx.enter_context(tc.tile_pool(name="xp", bufs=4))
        persist = ctx.enter_context(tc.tile_pool(name="persist", bufs=1))
        epool = ctx.enter_context(tc.tile_pool(name="ep", bufs=5))
        ypool = ctx.enter_context(tc.tile_pool(name="yp", bufs=2))
        ytpool = ctx.enter_context(tc.tile_pool(name="ytp", bufs=4))
        opool = ctx.enter_context(tc.tile_pool(name="op", bufs=3))
        small = ctx.enter_context(tc.tile_pool(name="sm", bufs=6))
        ps_big = ctx.enter_context(tc.tile_pool(name="psb", bufs=2, space="PSUM"))
        ps_y = ctx.enter_context(tc.tile_pool(name="psy", bufs=2, space="PSUM"))
        ps_o = ctx.enter_context(tc.tile_pool(name="pso", bufs=2, space="PSUM"))

        _x_tiles = {}

        def prefetch_x(n):
            if n not in _x_tiles and n < NCHUNK:
                th = xpool.tile([128, 2, KO2, CHUNK], F8, tag="x", name=f"xh{n}")
                nc.sync.dma_start(th[:], xh[n])
                tl = xpool.tile([128, 2, KO2, CHUNK], F8, tag="x", name=f"xl{n}")
                nc.sync.dma_start(tl[:], xl[n])
                _x_tiles[n] = (th, tl)

        def load_x(n):
            prefetch_x(n)
            return _x_tiles.pop(n)

        # ---- constants ----  (x chunk 0 is prefetched right after the wq
        # pair so the first projection matmul can start ~2.5us in)
        wq_sb, wk_sb, wv_sb = [], [], []
        _w_srcs = ((wq_sb, wqh, wql), (wk_sb, wkh, wkl), (wv_sb, wvh, wvl))
        _w_tiles = []
        for wn, (dst, hi, lo) in enumerate(_w_srcs):
            for hl, w_dram in enumerate((hi, lo)):
                t = const.tile([128, 2, KO2, DG], F8, name=f"w{wn}_{hl}")
                _w_tiles.append((t, w_dram))
                dst.append(t)
        _order = [0, 2, 1, 3, 4, 5]        # wq-hi, wk-hi, wq-lo, wk-lo, wv
        nc.sync.dma_start(_w_tiles[0][0][:], _w_tiles[0][1][:])   # wq hi
        nc.sync.dma_start(_w_tiles[2][0][:], _w_tiles[2][1][:])   # wk hi
        prefetch_x(0)
        for wi in (1, 3, 4, 5):
            t, w_dram = _w_tiles[wi]
            nc.sync.dma_start(t[:], w_dram[:])
        bq_sb = const.tile([128, 2], F32)
        nc.sync.dma_start(bq_sb[:], bq4[:])
        bk_sb = const.tile([128, 2], F32)
        nc.sync.dma_start(bk_sb[:], bk4[:])
        wpt_sb = const.tile([128, 2, C], BF16)
        nc.sync.dma_start(wpt_sb[:], wpt[:])

        ident = const.tile([128, 128], BF16)
        make_identity(nc, ident)
        ones_row = const.tile([1, 128], BF16)
        nc.vector.memset(ones_row[:], 1.0)
        zero_row = const.tile([1, 512], BF16)
        nc.vector.memset(zero_row[:], 0.0)
        # wmask[k, q] = 0 where q >= k else MASK_NEG (strict lower triangle
        # of keys over queries within the diagonal 128x128 block)
        wmask = const.tile([128, 128], BF16)
        nc.gpsimd.memset(wmask[:], 0.0)
        nc.gpsimd.affine_select(
            out=wmask[:],
            in_=wmask[:],
            compare_op=mybir.AluOpType.is_ge,
            fill=MASK_NEG,
            base=0,
            pattern=[[1, 128]],
            channel_multiplier=-1,
        )

        # ---- persistent activations ----
        # q8/k8: partition = (h%2)*64 + dh, free dims (pair, drslot, t).
        # drslot 1 is a constant zero operand: DoubleRow needs a [p, 2, n]
        # shape but the contraction is only 64 deep, so the second slot
        # multiplies zeros (and must be zeroed -- fp8 garbage can be NaN).
        q8 = persist.tile([128, 2, 2, T], F8)
        k8 = persist.tile([128, 2, 2, T], F8)
        nc.gpsimd.memset(q8[:, :, 1, :], 0.0)
        nc.gpsimd.memset(k8[:, :, 1, :], 0.0)
        # v: partition = key%128, free (ktile, head, dh+ones)
        v_sb = persist.tile([128, NKT, HPG, DH + 1], BF16)
        nc.vector.memset(v_sb[:, :, :, DH:DH + 1], 1.0)

        def proj_units(n):
            """Chunk-n projection emission as self-contained closures (one
            complete PSUM accumulation group each) so they can be spread
            across the previous chunk's exp-bound attention phase."""
            if n >= NCHUNK:
                return []
            cols = slice(n * CHUNK, (n + 1) * CHUNK)
            xs = {}

            def get_x():
                if "x" not in xs:
                    xs["x"] = load_x(n)
                    prefetch_x(n + 1)
                return xs["x"]

            units = []

            def qk_unit(w_pair, b_sb, dst, mt, tag):
                def run():
                    xthi, xtlo = get_x()
                    terms = ((xthi, 0), (xthi, 1), (xtlo, 0))
                    ps = ps_big.tile([128, 2, CHUNK], F32, tag="ps",
                                     name=f"p{tag}{n}_{mt}")
                    i, nmm = 0, len(terms) * KO2
                    for xt, wi in terms:
                        for ko in range(KO2):
                            nc.tensor.matmul(
                                ps[:, 0, :],
                                lhsT=w_pair[wi][:, :, ko, mt * 128:(mt + 1) * 128],
                                rhs=xt[:, :, ko, :],
                                start=(i == 0),
                                stop=(i == nmm - 1),
                                perf_mode=DR,
                            )
                            i += 1
                    nc.vector.tensor_scalar(
                        dst[:, mt, 0, cols], ps[:, 0, :],
                        SQ / (SX * SW), b_sb[:, mt:mt + 1],
                        op0=mybir.AluOpType.mult, op1=mybir.AluOpType.add,
                    )
                return run

            def v_unit(tt):
                def run():
                    xthi, xtlo = get_x()
                    terms = ((xthi, 0), (xthi, 1), (xtlo, 0))
                    kt = 4 * n + tt
                    ps = ps_big.tile([128, 2, CHUNK], F32, tag="ps",
                                     name=f"pv{n}_{tt}")
                    i, nmm = 0, len(terms) * KO2
                    for xt, wi in terms:
                        for ko in range(KO2):
                            nc.tensor.matmul(
                                ps[:, 0, 0:DG],
                                lhsT=xt[:, :, ko, tt * 128:(tt + 1) * 128],
                                rhs=wv_sb[wi][:, :, ko, :],
                                start=(i == 0),
                                stop=(i == nmm - 1),
                                perf_mode=DR,
                            )
                            i += 1
                    nc.vector.tensor_scalar_mul(
                        v_sb[:, kt, :, 0:DH], ps[:, 0, 0:DG], 1.0 / (SX * SW)
                    )
                return run

            for mt in range(2):
                units.append(qk_unit(wq_sb, bq_sb, q8, mt, "q"))
                units.append(qk_unit(wk_sb, bk_sb, k8, mt, "k"))
            for tt in range(4):
                units.append(v_unit(tt))
            return units

        DELAY = 3

        carry_out = []
        pre_scores = {}

        def emit_scores(sn, p, j):
            """Score matmuls + causal mask + exp for (chunk sn, pair p,
            key tile j); returns the bf16 e tile."""
            qlo = max(0, 128 * (j - 4 * sn))
            diag = j >= 4 * sn
            pss = ps_big.tile([128, 2, CHUNK], F32, tag="ps",
                              name=f"ss{sn}_{p}_{j}")
            for h01 in range(2):
                rows = slice(64 * h01, 64 * h01 + 64)
                nc.tensor.matmul(
                    pss[:, h01, qlo:],
                    lhsT=k8[rows, p, :, j * 128:(j + 1) * 128],
                    rhs=q8[rows, p, :,
                           sn * CHUNK + qlo:(sn + 1) * CHUNK],
                    start=True,
                    stop=not diag,
                    perf_mode=DR,
                )
            if diag:
                for h01 in range(2):
                    nc.tensor.matmul(
                        pss[:, h01, qlo:qlo + 128],
                        lhsT=ident[:],
                        rhs=wmask[:],
                        start=False,
                        stop=True,
                    )
            e = epool.tile([128, 2, CHUNK], BF16, tag="e")
            nc.scalar.activation(
                e[:, :, qlo:], pss[:, :, qlo:],
                mybir.ActivationFunctionType.Exp,
                scale=1.0 / (8.0 * SQ * SQ),
            )
            return e

        def attention(n, feed):
            """Scores + attn@v for chunk n (two head-pair passes). attn@v
            runs DELAY score-tiles behind the exp producing its input, and
            units from `feed` (next chunk's projection) are spread evenly
            over the score tiles to fill the PE while ACT works through
            the exps."""
            njt = 4 * (n + 1)
            steps_total = 2 * njt
            step_no = [0]
            nfeed = len(feed)
            consumed = [0]
            carried = list(carry_out)
            carry_out.clear()
            y_sb = ypool.tile([128, 4, HPG, DH], BF16, tag="y", name=f"y{n}")
            yt_sb = ytpool.tile([128, 2, CHUNK], BF16, tag="yt", name=f"yt{n}")
            pending_av = []
            pending_out = []

            def normalize(p, qt, psyA, psyB):
                rec = small.tile([128, 2], F32, tag="rec", name=f"rc{n}_{p}_{qt}")
                lv = small.tile([128, 2], F32, tag="lv", name=f"lv{n}_{p}_{qt}")
                nc.vector.tensor_copy(lv[:, 0:1], psyA[:, qt, DH:DH + 1])
                nc.vector.tensor_copy(lv[:, 1:2], psyB[:, qt, DH:DH + 1])
                nc.vector.reciprocal(rec[:], lv[:])
                for h01, psy in ((0, psyA), (1, psyB)):
                    nc.vector.tensor_scalar_mul(
                        y_sb[:, qt, 2 * p + h01, :], psy[:, qt, 0:DH],
                        rec[:, h01:h01 + 1],
                    )

            def transpose_y(qt):
                # y[q, ydim] -> yT[ydim, t] via the PE transpose datapath;
                # output lands bf16 in a bitcast corner of an outproj-pool
                # tile (keeps the scores-pool rotation free of chunk-tail
                # eviction dependencies)
                tp = ps_big.tile([128, 2, CHUNK], F32, tag="ps", name=f"tp{n}_{qt}")
                tpb = tp[:].bitcast(BF16)
                for mt in range(2):
                    nc.tensor.matmul(
                        tpb[:, mt, 0:128],
                        lhsT=y_sb[:, qt, 2 * mt:2 * mt + 2, :],
                        rhs=ident[:],
                        is_transpose=True,
                    )
                nc.vector.tensor_copy(
                    yt_sb[:, :, qt * 128:(qt + 1) * 128], tpb[:, :, 0:128]
                )

            def outproj(qt):
                t_tile = 4 * n + qt
                o_sb = opool.tile([128, C], BF16, tag="o", name=f"o{n}_{qt}")
                for nh in range(2):
                    ps = ps_o.tile([128, 512], F32, tag="o", name=f"po{n}_{qt}_{nh}")
                    for mt in range(2):
                        nc.tensor.matmul(
                            ps[:],
                            lhsT=yt_sb[:, mt, qt * 128:(qt + 1) * 128],
                            rhs=wpt_sb[:, mt, nh * 512:(nh + 1) * 512],
                            start=(mt == 0),
                            stop=(mt == 1),
                        )
                    nc.vector.tensor_copy(o_sb[:, nh * 512:(nh + 1) * 512], ps[:])
                    nc.sync.dma_start(
                        out[t_tile][:, nh * 512:(nh + 1) * 512],
                        o_sb[:, nh * 512:(nh + 1) * 512],
                    )

            def attnv(p, j, e, box):
                def run():
                    psyA, psyB = box["A"], box["B"]
                    qlo = max(0, 128 * (j - 4 * n))
                    for qt in range(qlo // 128, 4):
                        for h01, psy in ((0, psyA), (1, psyB)):
                            nc.tensor.matmul(
                                psy[:, qt, 0:DH + 1],
                                lhsT=e[:, h01, qt * 128:(qt + 1) * 128],
                                rhs=v_sb[:, j, 2 * p + h01, :],
                                start=False,
                                stop=(j == 4 * n + qt),
                                skip_group_check=True,
                            )
                    if j >= 4 * n:
                        qt_done = j - 4 * n
                        normalize(p, qt_done, psyA, psyB)

                        if p == 1:
                            transpose_y(qt_done)
                            if n < NCHUNK - 1:
                                carry_out.append(
                                    lambda qt=qt_done: outproj(qt))
                            else:
                                pending_out.append(qt_done)
                                if len(pending_out) > 1:
                                    outproj(pending_out.pop(0))
                return run

            def step():
                if pending_av and len(pending_av) > DELAY:
                    pending_av.pop(0)()
                step_no[0] += 1
                if carried and step_no[0] % 2 == 0:
                    carried.pop(0)()
                # spread feed units so they finish ~7/8 through the chunk
                if feed:
                    target = min(len(feed) + consumed[0],
                                 (nfeed * 8 * step_no[0])
                                 // (7 * steps_total) + 1)
                    while consumed[0] < target and feed:
                        feed.pop(0)()
                        consumed[0] += 1

            for p in range(2):
                box = {}
                for j in range(njt):
                    e = pre_scores.pop((n, p, j), None)
                    if e is None:
                        e = emit_scores(n, p, j)
                    if j == 1:
                        # allocate + zero the psy banks only now: emitted
                        # after the pass's first scores, so the PE isn't
                        # stalled on the previous pass's last normalize
                        # (the WAR dependency of the bank reuse)
                        for key in ("A", "B"):
                            psy = ps_y.tile([128, 4, 128], F32, tag="psy",
                                            name=f"py{key}{n}_{p}")
                            # one start per PSUM bank: a K=1 matmul zeroes
                            # the bank so every attn@v slot accumulates with
                            # start=False (multiple start groups in one 2KB
                            # region clobber siblings)
                            nc.tensor.matmul(
                                psy[:, :, :].rearrange("p a b -> p (a b)"),
                                lhsT=ones_row[:],
                                rhs=zero_row[:],
                                start=True,
                                stop=True,
                                skip_group_check=True,
                            )
                            box[key] = psy
                    if p == 1 and j == njt - 1 and n + 1 < NCHUNK:
                        # pre-emit the next chunk's first two score tiles so
                        # ACT streams straight through the chunk boundary
                        for jj in range(2):
                            pre_scores[(n + 1, 0, jj)] = emit_scores(
                                n + 1, 0, jj)
                    pending_av.append(attnv(p, j, e, box))
                    step()
            while pending_av:
                pending_av.pop(0)()
                while feed:
                    feed.pop(0)()
            for fn in carried:
                fn()
            if n < NCHUNK - 1:
                for qt in pending_out:
                    carry_out.append(lambda qt=qt: outproj(qt))
            else:
                for qt in pending_out:
                    outproj(qt)

        for u in proj_units(0):
            u()
        for n in range(NCHUNK):
            feed = proj_units(n + 1)
            attention(n, feed)
            for u in feed:
                u()

    return nc


_NC_CACHE = None


def _split8(a, s):
    hi = np.asarray(a * s, NPF8)
    lo = np.asarray(a * s - hi.astype(np.float32), NPF8)
    return hi, lo


def kernel(**inputs) -> np.ndarray:
    global _NC_CACHE
    x = np.asarray(inputs["x"], np.float32)
    Wq = np.asarray(inputs["Wq"], np.float32)
    Wk = np.asarray(inputs["Wk"], np.float32)
    Wv = np.asarray(inputs["Wv"], np.float32)
    Wp = np.asarray(inputs["Wp"], np.float32)
    bq = np.asarray(inputs["bq"], np.float32)
    bk = np.asarray(inputs["bk"], np.float32)
    bv = np.asarray(inputs["bv"], np.float32)
    bp = np.asarray(inputs["bp"], np.float32)

    if _NC_CACHE is None:
        _NC_CACHE = build_kernel()
    nc = _NC_CACHE

    def pack_w(Wl):
        # Wl: [256 out, 1024 in] slice -> lhsT [c, m] -> [p, slot, ko, m]
        wt = Wl.T                                          # [1024 c, 256 m]
        wt = wt.reshape(KO2, 2, 128, 256).transpose(2, 1, 0, 3)
        hi, lo = _split8(np.ascontiguousarray(wt), SW)
        return np.ascontiguousarray(hi), np.ascontiguousarray(lo)

    in_maps = []
    for c in range(NCORES):
        b, g = divmod(c, GROUPS)
        rows = slice(g * DG, (g + 1) * DG)
        xt = x[b].T.reshape(KO2, 2, 128, T).transpose(2, 1, 0, 3)  # [p,slot,ko,t]
        xt = xt.reshape(128, 2, KO2, NCHUNK, CHUNK).transpose(3, 0, 1, 2, 4)
        xhi, xlo = _split8(np.ascontiguousarray(xt), SX)

        wq_hi, wq_lo = pack_w(Wq[rows])
        wk_hi, wk_lo = pack_w(Wk[rows])
        wv_hi, wv_lo = pack_w(Wv[rows])
        wpt_l = np.ascontiguousarray(
            Wp[:, rows].T.reshape(2, 128, C).transpose(1, 0, 2)
        ).astype(NPBF16)

        bq4 = np.ascontiguousarray((bq[rows] * SQ).reshape(2, 128).T)
        bk4 = np.ascontiguousarray((bk[rows] * SQ).reshape(2, 128).T)

        in_maps.append({
            "xh": np.ascontiguousarray(xhi),
            "xl": np.ascontiguousarray(xlo),
            "wqh": wq_hi, "wql": wq_lo,
            "wkh": wk_hi, "wkl": wk_lo,
            "wvh": wv_hi, "wvl": wv_lo,
            "wpt": wpt_l,
            "bq4": bq4, "bk4": bk4,
        })

    res = run_bass_kernel_spmd(nc, in_maps, core_ids=list(range(NCORES)))

    result = np.zeros((B, T, C), np.float32)
    for c in range(NCORES):
        b = c // GROUPS
        o = np.asarray(res.results[c]["out"]).astype(np.float32)
        result[b] += o.reshape(T, C)
    result += (bv @ Wp.T + bp)[None, None, :]
    return result


# revision 50
# speedup vs baseline: 1.0087x; 1.0024x over previous
"""Causal self-attention on 8 trn2 NeuronCores.

Sharding: core c -> (batch b = c // 4, head-group g = c % 4). Each core
computes 4 of the 16 heads for one batch element plus its slice of the
output projection; the host sums the 4 partial projections per batch and
adds the constant (bv @ Wp.T + bp) term exactly.

Kernel structure (per core), streamed over 4 query chunks of 512:
  - Q/K/V projections as fp8e4 DoubleRow matmuls with hi+lo residual
    splits of both x and W (3 accumulation terms; quantization error
    ~0.05%), contraction 256/step.
  - Scores s = k.T q in fp8e4 DoubleRow ([keys, queries] orientation,
    dh packed 32x2), causal mask added in PSUM via an identity matmul,
    exp on the Activation engine straight out of PSUM into bf16 SBUF.
  - attn@v flipped: e is the stationary operand, v (with a trailing
    ones column that accumulates the softmax denominator l) is moving;
    PSUM rows are queries so 1/l is a per-partition scalar folded into
    the eviction tensor_scalar op.
  - y transposed via the DMA xbar (16x128 tiles) into [ydim, t] layout,
    then the output projection in bf16; out partials stored bf16.
"""

import numpy as np
import ml_dtypes

import concourse.bass as bass
import concourse.mybir as mybir
import concourse.tile as tile
from concourse.bass_utils import run_bass_kernel_spmd

B = 2
T = 2048
C = 1024
H = 16
DH = 64
NCORES = 8
GROUPS = 4            # head groups (tensor parallel)
HPG = H // GROUPS     # heads per group = 4
DG = HPG * DH         # head-group width = 256
CHUNK = 512           # query-chunk size
NCHUNK = T // CHUNK   # 4
KO2 = C // 256        # DoubleRow contraction steps for the projections
NKT = T // 128        # key tiles
F32 = mybir.dt.float32
F32R = mybir.dt.float32r
BF16 = mybir.dt.bfloat16
F8 = mybir.dt.float8e4
NPF8 = ml_dtypes.float8_e4m3
NPBF16 = ml_dtypes.bfloat16
MASK_NEG = -1e30

SX = 16.0             # x fp8 scale
SW = 256.0            # weight fp8 scale
SQ = 4.0              # q/k fp8 store scale
DR = mybir.MatmulPerfMode.DoubleRow


def _patch_tile_drain():
    """This walrus build lowers Drain/NOP to a CTRL with a single sync-wait
    slot; TileContext's kernel-tail drain accumulates one wait per live
    semaphore and fails codegen. Split the waits across single-wait NOPs."""
    import bass_rust
    from concourse.tile import TileContext

    def _drain_and_barrier_split(self, tick_clock, wait_clock):
        probe = self.nc.sync.nop()
        wait_clock.add_sem_waits(
            probe.ins, tile.ScopedClock({None: tick_clock.global_clock})
        )
        waits = list(probe.ins.sync_info.on_wait or [])
        probe.ins.sync_info.on_wait = []
        engines = [self.nc.sync, self.nc.tensor, self.nc.vector,
                   self.nc.scalar, self.nc.gpsimd]
        for i, w in enumerate(waits):
            n = engines[i % len(engines)].nop()
            if n.ins.sync_info is None:
                n.ins.sync_info = bass_rust.SyncInfo(on_wait=[w], on_update=[])
            else:
                n.ins.sync_info.on_wait = [w]
        self.nc.sync.drain()
        self.nc.all_engine_barrier()
        assert self.sems is not None
        popped = self.nc._tile_sem_poison_stack.pop()
        assert popped is self._sem_poison
        self.nc.clear_and_free_semaphores(list(self.sems.allocated().values()))
        self.nc.all_engine_barrier()

    TileContext._drain_and_barrier = _drain_and_barrier_split

    import json as _json

    import concourse.bass2jax as bass2jax
    import concourse.bass_utils as bass_utils

    if getattr(bass_utils.compile_bir_kernel, "_wait_split", False):
        return

    _orig_compile = bass_utils.compile_bir_kernel

    def _split_multi_waits(bir_json):
        m = _json.loads(bir_json)
        counter = 0
        changed = False
        for fn in m["functions"]:
            for blk in fn["blocks"]:
                new_insts = []
                for inst in blk["instructions"]:
                    si = inst.get("sync_info")
                    waits = (si or {}).get("on_wait") or []
                    sem_waits = [w for w in waits if w.get("sync_type") == "semaphore"]
                    if len(waits) > 1 and len(sem_waits) == len(waits):
                        changed = True
                        for w in waits[:-1]:
                            counter += 1
                            new_insts.append({
                                "name": f"I-wsplit{counter}",
                                "opcode": "NoOp",
                                "engine": inst["engine"],
                                "ins": [],
                                "outs": [],
                                "sync_info": {"on_wait": [w], "on_update": []},
                            })
                        si["on_wait"] = [waits[-1]]
                    new_insts.append(inst)
                blk["instructions"] = new_insts
        if not changed:
            return bir_json
        return _json.dumps(m).encode()

    def _compile_bir_kernel_split(bir_json, tmpdir, neff_name="file.neff"):
        return _orig_compile(_split_multi_waits(bir_json), tmpdir, neff_name=neff_name)

    _compile_bir_kernel_split._wait_split = True
    bass_utils.compile_bir_kernel = _compile_bir_kernel_split
    bass2jax.compile_bir_kernel = _compile_bir_kernel_split


def build_kernel():
    _patch_tile_drain()
    nc = bass.Bass(target_bir_lowering=False, trn_type="TRN2")

    # hi/lo fp8 operand pairs; layouts are DoubleRow-packed on the host:
    # contraction index c = ko*256 + slot*128 + p.
    xh = nc.dram_tensor("xh", [NCHUNK, 128, 2, KO2, CHUNK], F8, kind="ExternalInput")
    xl = nc.dram_tensor("xl", [NCHUNK, 128, 2, KO2, CHUNK], F8, kind="ExternalInput")
    wqh = nc.dram_tensor("wqh", [128, 2, KO2, DG], F8, kind="ExternalInput")
    wql = nc.dram_tensor("wql", [128, 2, KO2, DG], F8, kind="ExternalInput")
    wkh = nc.dram_tensor("wkh", [128, 2, KO2, DG], F8, kind="ExternalInput")
    wkl = nc.dram_tensor("wkl", [128, 2, KO2, DG], F8, kind="ExternalInput")
    wvh = nc.dram_tensor("wvh", [128, 2, KO2, DG], F8, kind="ExternalInput")
    wvl = nc.dram_tensor("wvl", [128, 2, KO2, DG], F8, kind="ExternalInput")
    wpt = nc.dram_tensor("wpt", [128, 2, C], BF16, kind="ExternalInput")
    bq4 = nc.dram_tensor("bq4", [128, 2], F32, kind="ExternalInput")
    bk4 = nc.dram_tensor("bk4", [128, 2], F32, kind="ExternalInput")
    out = nc.dram_tensor("out", [NKT, 128, C], BF16, kind="ExternalOutput")

    from contextlib import ExitStack

    with tile.TileContext(nc) as tc, ExitStack() as ctx:
        from concourse.masks import make_identity

        const = ctx.enter_context(tc.tile_pool(name="const", bufs=1))
        xpool = ctx.enter_context(tc.tile_pool(name="xp", bufs=4))
        persist = ctx.enter_context(tc.tile_pool(name="persist", bufs=1))
        epool = ctx.enter_context(tc.tile_pool(name="ep", bufs=6))
        ypool = ctx.enter_context(tc.tile_pool(name="yp", bufs=2))
        ytpool = ctx.enter_context(tc.tile_pool(name="ytp", bufs=4))
        opool = ctx.enter_context(tc.tile_pool(name="op", bufs=3))
        small = ctx.enter_context(tc.tile_pool(name="sm", bufs=6))
        ps_big = ctx.enter_context(tc.tile_pool(name="psb", bufs=2, space="PSUM"))
        ps_y = ctx.enter_context(tc.tile_pool(name="psy", bufs=2, space="PSUM"))
        ps_o = ctx.enter_context(tc.tile_pool(name="pso", bufs=2, space="PSUM"))

        _x_tiles = {}

        def prefetch_x(n):
            if n not in _x_tiles and n < NCHUNK:
                th = xpool.tile([128, 2, KO2, CHUNK], F8, tag="x", name=f"xh{n}")
                nc.sync.dma_start(th[:], xh[n])
                tl = xpool.tile([128, 2, KO2, CHUNK], F8, tag="x", name=f"xl{n}")
                nc.sync.dma_start(tl[:], xl[n])
                _x_tiles[n] = (th, tl)

        def load_x(n):
            prefetch_x(n)
            return _x_tiles.pop(n)

        # ---- constants ----  (x chunk 0 is prefetched right after the wq
        # pair so the first projection matmul can start ~2.5us in)
        wq_sb, wk_sb, wv_sb = [], [], []
        _w_srcs = ((wq_sb, wqh, wql), (wk_sb, wkh, wkl), (wv_sb, wvh, wvl))
        _w_tiles = []
        for wn, (dst, hi, lo) in enumerate(_w_srcs):
            for hl, w_dram in enumerate((hi, lo)):
                t = const.tile([128, 2, KO2, DG], F8, name=f"w{wn}_{hl}")
                _w_tiles.append((t, w_dram))
                dst.append(t)
        _order = [0, 2, 1, 3, 4, 5]        # wq-hi, wk-hi, wq-lo, wk-lo, wv
        nc.sync.dma_start(_w_tiles[0][0][:], _w_tiles[0][1][:])   # wq hi
        nc.sync.dma_start(_w_tiles[2][0][:], _w_tiles[2][1][:])   # wk hi
        prefetch_x(0)
        for wi in (1, 3, 4, 5):
            t, w_dram = _w_tiles[wi]
            nc.sync.dma_start(t[:], w_dram[:])
        bq_sb = const.tile([128, 2], F32)
        nc.sync.dma_start(bq_sb[:], bq4[:])
        bk_sb = const.tile([128, 2], F32)
        nc.sync.dma_start(bk_sb[:], bk4[:])
        wpt_sb = const.tile([128, 2, C], BF16)
        nc.sync.dma_start(wpt_sb[:], wpt[:])

        ident = const.tile([128, 128], BF16)
        make_identity(nc, ident)
        ones_row = const.tile([1, 128], BF16)
        nc.vector.memset(ones_row[:], 1.0)
        zero_row = const.tile([1, 512], BF16)
        nc.vector.memset(zero_row[:], 0.0)
        # wmask[k, q] = 0 where q >= k else MASK_NEG (strict lower triangle
        # of keys over queries within the diagonal 128x128 block)
        wmask = const.tile([128, 128], BF16)
        nc.gpsimd.memset(wmask[:], 0.0)
        nc.gpsimd.affine_select(
            out=wmask[:],
            in_=wmask[:],
            compare_op=mybir.AluOpType.is_ge,
            fill=MASK_NEG,
            base=0,
            pattern=[[1, 128]],
            channel_multiplier=-1,
        )

        # ---- persistent activations ----
        # q8/k8: partition = (h%2)*64 + dh, free dims (pair, drslot, t).
        # drslot 1 is a constant zero operand: DoubleRow needs a [p, 2, n]
        # shape but the contraction is only 64 deep, so the second slot
        # multiplies zeros (and must be zeroed -- fp8 garbage can be NaN).
        q8 = persist.tile([128, 2, 2, T], F8)
        k8 = persist.tile([128, 2, 2, T], F8)
        nc.gpsimd.memset(q8[:, :, 1, :], 0.0)
        nc.gpsimd.memset(k8[:, :, 1, :], 0.0)
        # v: partition = key%128, free (ktile, head, dh+ones)
        v_sb = persist.tile([128, NKT, HPG, DH + 1], BF16)
        nc.vector.memset(v_sb[:, :, :, DH:DH + 1], 1.0)

        def proj_units(n):
            """Chunk-n projection emission as self-contained closures (one
            complete PSUM accumulation group each) so they can be spread
            across the previous chunk's exp-bound attention phase."""
            if n >= NCHUNK:
                return []
            cols = slice(n * CHUNK, (n + 1) * CHUNK)
            xs = {}

            def get_x():
                if "x" not in xs:
                    xs["x"] = load_x(n)
                    prefetch_x(n + 1)
                return xs["x"]

            units = []

            def qk_unit(w_pair, b_sb, dst, mt, tag):
                def run():
                    xthi, xtlo = get_x()
                    terms = ((xthi, 0), (xthi, 1), (xtlo, 0))
                    ps = ps_big.tile([128, 2, CHUNK], F32, tag="ps",
                                     name=f"p{tag}{n}_{mt}")
                    i, nmm = 0, len(terms) * KO2
                    for xt, wi in terms:
                        for ko in range(KO2):
                            nc.tensor.matmul(
                                ps[:, 0, :],
                                lhsT=w_pair[wi][:, :, ko, mt * 128:(mt + 1) * 128],
                                rhs=xt[:, :, ko, :],
                                start=(i == 0),
                                stop=(i == nmm - 1),
                                perf_mode=DR,
                            )
                            i += 1
                    nc.vector.tensor_scalar(
                        dst[:, mt, 0, cols], ps[:, 0, :],
                        SQ / (SX * SW), b_sb[:, mt:mt + 1],
                        op0=mybir.AluOpType.mult, op1=mybir.AluOpType.add,
                    )
                return run

            def v_unit(tt):
                def run():
                    xthi, xtlo = get_x()
                    terms = ((xthi, 0), (xthi, 1), (xtlo, 0))
                    kt = 4 * n + tt
                    ps = ps_big.tile([128, 2, CHUNK], F32, tag="ps",
                                     name=f"pv{n}_{tt}")
                    i, nmm = 0, len(terms) * KO2
                    for xt, wi in terms:
                        for ko in range(KO2):
                            nc.tensor.matmul(
                                ps[:, 0, 0:DG],
                                lhsT=xt[:, :, ko, tt * 128:(tt + 1) * 128],
                                rhs=wv_sb[wi][:, :, ko, :],
                                start=(i == 0),
                                stop=(i == nmm - 1),
                                perf_mode=DR,
                            )
                            i += 1
                    nc.vector.tensor_scalar_mul(
                        v_sb[:, kt, :, 0:DH], ps[:, 0, 0:DG], 1.0 / (SX * SW)
                    )
                return run

            for mt in range(2):
                units.append(qk_unit(wq_sb, bq_sb, q8, mt, "q"))
                units.append(qk_unit(wk_sb, bk_sb, k8, mt, "k"))
            for tt in range(4):
                units.append(v_unit(tt))
            return units

        DELAY = 3

        carry_out = []
        pre_scores = {}

        def emit_scores(sn, p, j):
            """Score matmuls + causal mask + exp for (chunk sn, pair p,
            key tile j); returns the bf16 e tile."""
            qlo = max(0, 128 * (j - 4 * sn))
            diag = j >= 4 * sn
            pss = ps_big.tile([128, 2, CHUNK], F32, tag="ps",
                              name=f"ss{sn}_{p}_{j}")
            for h01 in range(2):
                rows = slice(64 * h01, 64 * h01 + 64)
                nc.tensor.matmul(
                    pss[:, h01, qlo:],
                    lhsT=k8[rows, p, :, j * 128:(j + 1) * 128],
                    rhs=q8[rows, p, :,
                           sn * CHUNK + qlo:(sn + 1) * CHUNK],
                    start=True,
                    stop=not diag,
                    perf_mode=DR,
                )
            if diag:
                for h01 in range(2):
                    nc.tensor.matmul(
                        pss[:, h01, qlo:qlo + 128],
                        lhsT=ident[:],
                        rhs=wmask[:],
                        start=False,
                        stop=True,
                    )
            e = epool.tile([128, 2, CHUNK], BF16, tag="e")
            nc.scalar.activation(
                e[:, :, qlo:], pss[:, :, qlo:],
                mybir.ActivationFunctionType.Exp,
                scale=1.0 / (8.0 * SQ * SQ),
            )
            return e

        def attention(n, feed):
            """Scores + attn@v for chunk n (two head-pair passes). attn@v
            runs DELAY score-tiles behind the exp producing its input, and
            units from `feed` (next chunk's projection) are spread evenly
            over the score tiles to fill the PE while ACT works through
            the exps."""
            njt = 4 * (n + 1)
            steps_total = 2 * njt
            step_no = [0]
            nfeed = len(feed)
            consumed = [0]
            carried = list(carry_out)
            carry_out.clear()
            y_sb = ypool.tile([128, 4, HPG, DH], BF16, tag="y", name=f"y{n}")
            yt_sb = ytpool.tile([128, 2, CHUNK], BF16, tag="yt", name=f"yt{n}")
            pending_av = []
            pending_out = []

            def normalize(p, qt, psyA, psyB):
                rec = small.tile([128, 2], F32, tag="rec", name=f"rc{n}_{p}_{qt}")
                lv = small.tile([128, 2], F32, tag="lv", name=f"lv{n}_{p}_{qt}")
                nc.vector.tensor_copy(lv[:, 0:1], psyA[:, qt, DH:DH + 1])
                nc.vector.tensor_copy(lv[:, 1:2], psyB[:, qt, DH:DH + 1])
                nc.vector.reciprocal(rec[:], lv[:])
                for h01, psy in ((0, psyA), (1, psyB)):
                    nc.vector.tensor_scalar_mul(
                        y_sb[:, qt, 2 * p + h01, :], psy[:, qt, 0:DH],
                        rec[:, h01:h01 + 1],
                    )

            def transpose_y(qt):
                # y[q, ydim] -> yT[ydim, t] via the PE transpose datapath;
                # output lands bf16 in a bitcast corner of an outproj-pool
                # tile (keeps the scores-pool rotation free of chunk-tail
                # eviction dependencies)
                tp = ps_big.tile([128, 2, CHUNK], F32, tag="ps", name=f"tp{n}_{qt}")
                tpb = tp[:].bitcast(BF16)
                for mt in range(2):
                    nc.tensor.matmul(
                        tpb[:, mt, 0:128],
                        lhsT=y_sb[:, qt, 2 * mt:2 * mt + 2, :],
                        rhs=ident[:],
                        is_transpose=True,
                    )
                nc.vector.tensor_copy(
                    yt_sb[:, :, qt * 128:(qt + 1) * 128], tpb[:, :, 0:128]
                )

            def outproj(qt):
                t_tile = 4 * n + qt
                o_sb = opool.tile([128, C], BF16, tag="o", name=f"o{n}_{qt}")
                for nh in range(2):
                    ps = ps_o.tile([128, 512], F32, tag="o", name=f"po{n}_{qt}_{nh}")
                    for mt in range(2):
                        nc.tensor.matmul(
                            ps[:],
                            lhsT=yt_sb[:, mt, qt * 128:(qt + 1) * 128],
                            rhs=wpt_sb[:, mt, nh * 512:(nh + 1) * 512],
                            start=(mt == 0),
                            stop=(mt == 1),
                        )
                    nc.vector.tensor_copy(o_sb[:, nh * 512:(nh + 1) * 512], ps[:])
                    nc.sync.dma_start(
                        out[t_tile][:, nh * 512:(nh + 1) * 512],
                        o_sb[:, nh * 512:(nh + 1) * 512],
                    )

            def attnv(p, j, e, box):
                def run():
                    psyA, psyB = box["A"], box["B"]
                    qlo = max(0, 128 * (j - 4 * n))
                    for qt in range(qlo // 128, 4):
                        for h01, psy in ((0, psyA), (1, psyB)):
                            nc.tensor.matmul(
                                psy[:, qt, 0:DH + 1],
                                lhsT=e[:, h01, qt * 128:(qt + 1) * 128],
                                rhs=v_sb[:, j, 2 * p + h01, :],
                                start=False,
                                stop=(j == 4 * n + qt),
                                skip_group_check=True,
                            )
                    if j >= 4 * n:
                        qt_done = j - 4 * n
                        normalize(p, qt_done, psyA, psyB)

                        if p == 1:
                            transpose_y(qt_done)
                            if n < NCHUNK - 1:
                                carry_out.append(
                                    lambda qt=qt_done: outproj(qt))
                            else:
                                pending_out.append(qt_done)
                                if len(pending_out) > 1:
                                    outproj(pending_out.pop(0))
                return run

            def step():
                if pending_av and len(pending_av) > DELAY:
                    pending_av.pop(0)()
                step_no[0] += 1
                if carried and step_no[0] % 2 == 0:
                    carried.pop(0)()
                # spread feed units so they finish ~7/8 through the chunk
                if feed:
                    target = min(len(feed) + consumed[0],
                                 (nfeed * 8 * step_no[0])
                                 // (7 * steps_total) + 1)
                    while consumed[0] < target and feed:
                        feed.pop(0)()
                        consumed[0] += 1

            for p in range(2):
                box = {}
                for j in range(njt):
                    e = pre_scores.pop((n, p, j), None)
                    if e is None:
                        e = emit_scores(n, p, j)
                    if j == 1:
                        # allocate + zero the psy banks only now: emitted
                        # after the pass's first scores, so the PE isn't
                        # stalled on the previous pass's last normalize
                        # (the WAR dependency of the bank reuse)
                        for key in ("A", "B"):
                            psy = ps_y.tile([128, 4, 128], F32, tag="psy",
                                            name=f"py{key}{n}_{p}")
                            # one start per PSUM bank: a K=1 matmul zeroes
                            # the bank so every attn@v slot accumulates with
                            # start=False (multiple start groups in one 2KB
                            # region clobber siblings)
                            nc.tensor.matmul(
                                psy[:, :, :].rearrange(
                                    "p a b -> p (a b)")[:, 0:449],
                                lhsT=ones_row[:],
                                rhs=zero_row[:, 0:449],
                                start=True,
                                stop=True,
                                skip_group_check=True,
                            )
                            box[key] = psy
                    if p == 1 and j == njt - 1 and n + 1 < NCHUNK:
                        # pre-emit the next chunk's first two score tiles so
                        # ACT streams straight through the chunk boundary
                        for jj in range(2):
                            pre_scores[(n + 1, 0, jj)] = emit_scores(
                                n + 1, 0, jj)
                    pending_av.append(attnv(p, j, e, box))
                    step()
            while pending_av:
                pending_av.pop(0)()
                while feed:
                    feed.pop(0)()
            for fn in carried:
                fn()
            if n < NCHUNK - 1:
                for qt in pending_out:
                    carry_out.append(lambda qt=qt: outproj(qt))
            else:
                for qt in pending_out:
                    outproj(qt)

        for u in proj_units(0):
            u()
        for n in range(NCHUNK):
            feed = proj_units(n + 1)
            attention(n, feed)
            for u in feed:
                u()

    return nc


_NC_CACHE = None


def _split8(a, s):
    hi = np.asarray(a * s, NPF8)
    lo = np.asarray(a * s - hi.astype(np.float32), NPF8)
    return hi, lo


def kernel(**inputs) -> np.ndarray:
    global _NC_CACHE
    x = np.asarray(inputs["x"], np.float32)
    Wq = np.asarray(inputs["Wq"], np.float32)
    Wk = np.asarray(inputs["Wk"], np.float32)
    Wv = np.asarray(inputs["Wv"], np.float32)
    Wp = np.asarray(inputs["Wp"], np.float32)
    bq = np.asarray(inputs["bq"], np.float32)
    bk = np.asarray(inputs["bk"], np.float32)
    bv = np.asarray(inputs["bv"], np.float32)
    bp = np.asarray(inputs["bp"], np.float32)

    if _NC_CACHE is None:
        _NC_CACHE = build_kernel()
    nc = _NC_CACHE

    def pack_w(Wl):
        # Wl: [256 out, 1024 in] slice -> lhsT [c, m] -> [p, slot, ko, m]
        wt = Wl.T                                          # [1024 c, 256 m]
        wt = wt.reshape(KO2, 2, 128, 256).transpose(2, 1, 0, 3)
        hi, lo = _split8(np.ascontiguousarray(wt), SW)
        return np.ascontiguousarray(hi), np.ascontiguousarray(lo)

    in_maps = []
    for c in range(NCORES):
        b, g = divmod(c, GROUPS)
        rows = slice(g * DG, (g + 1) * DG)
        xt = x[b].T.reshape(KO2, 2, 128, T).transpose(2, 1, 0, 3)  # [p,slot,ko,t]
        xt = xt.reshape(128, 2, KO2, NCHUNK, CHUNK).transpose(3, 0, 1, 2, 4)
        xhi, xlo = _split8(np.ascontiguousarray(xt), SX)

        wq_hi, wq_lo = pack_w(Wq[rows])
        wk_hi, wk_lo = pack_w(Wk[rows])
        wv_hi, wv_lo = pack_w(Wv[rows])
        wpt_l = np.ascontiguousarray(
            Wp[:, rows].T.reshape(2, 128, C).transpose(1, 0, 2)
        ).astype(NPBF16)

        bq4 = np.ascontiguousarray((bq[rows] * SQ).reshape(2, 128).T)
        bk4 = np.ascontiguousarray((bk[rows] * SQ).reshape(2, 128).T)

        in_maps.append({
            "xh": np.ascontiguousarray(xhi),
            "xl": np.ascontiguousarray(xlo),
            "wqh": wq_hi, "wql": wq_lo,
            "wkh": wk_hi, "wkl": wk_lo,
            "wvh": wv_hi, "wvl": wv_lo,
            "wpt": wpt_l,
            "bq4": bq4, "bk4": bk4,
        })

    res = run_bass_kernel_spmd(nc, in_maps, core_ids=list(range(NCORES)))

    result = np.zeros((B, T, C), np.float32)
    for c in range(NCORES):
        b = c // GROUPS
        o = np.asarray(res.results[c]["out"]).astype(np.float32)
        result[b] += o.reshape(T, C)
    result += (bv @ Wp.T + bp)[None, None, :]
    return result
